# revision 1
# baseline (speedup 1.0000x reference)
"""Trainium2 Bass kernel for nn_Attention_18631568130798.

Mixed template/search attention (Stark-style tracker attention):
  qkv proj -> per-head scores + RPE bias -> template-block softmax ->
  boxmask-weighted factor -> rescaled template->search scores -> softmax ->
  attn @ v -> output proj.

Sharding: data-parallel over batch B=64 across 8 NeuronCores (8 batches/core).

v2 design (vs f32r baseline):
  * all-bf16 dataflow (PE matmuls run 1 cycle/row at ANY free size; DVE ops
    at 2x 16-bit rate; half the SBUF/DMA traffic). Host numpy estimate of
    the full-bf16 pipeline error: 4.4e-3 relmax (gate 2e-2).
  * RPE bias gather done on HOST (pure function of rpe_table/rpe_index
    inputs) and DMA'd as a ready bf16 [j, h, i] table - kills the gpsimd
    gather + DRAM bounce preamble.
  * ones-row appended to v and bm lhsT tiles: attention @ v matmuls emit
    softmax denominators as an extra output row - no separate denominator
    matmuls.
  * per-batch-PAIR processing with double-buffered tile pools so consecutive
    pairs/batches overlap across engines.
  * scores are computed TRANSPOSED: ST[j, i] = k_j . q_i, making the
    out^T = (c-part, i-free) of attn@v exactly the lhsT the output
    projection needs. No PE transposes anywhere.
"""

import os
import numpy as np
import ml_dtypes

import concourse.bass as bass
import concourse.mybir as mybir
import concourse.tile as tile
from concourse import bacc
from concourse.bass_utils import run_bass_kernel_spmd

AF = mybir.ActivationFunctionType
F32 = mybir.dt.float32
BF16 = mybir.dt.bfloat16
BF = ml_dtypes.bfloat16

# Problem constants (hardcoded per contract)
B, N, C = 64, 245, 768
H, HD = 12, 64
NT, NS = 49, 196
SCALE = HD ** -0.5
NCORES = 8
NB = B // NCORES              # batches per core
JCH = [(0, 128), (128, 117)]  # j (key-token) partition chunks
# 3-head psum groups with constant partition-half parity s=h%2; parity
# alternates so head-pairs (2cc, 2cc+1) complete in cc order for the
# output projection's accumulation stream
HGROUPS = [(0, [0, 2, 4]), (0, [1, 3, 5]), (3, [6, 8, 10]), (3, [7, 9, 11])]


# variant switches for A/B experiments (read at build time; key in _CACHE)
VARIANT = {"attnv_pbcast": True, "zero_bias": False}


def _build(nb: int, reps: int = 1):
    nc = bacc.Bacc(None, target_bir_lowering=False, name="attn18631v2")

    xT_d = nc.dram_tensor("xT", [nb, C, N], BF16, kind="ExternalInput")
    bm_d = nc.dram_tensor("bm", [nb, NT, H, HD], BF16, kind="ExternalInput")
    wqkT_d = nc.dram_tensor("wqkT", [C, 2 * C], BF16, kind="ExternalInput")
    wvT_d = nc.dram_tensor("wvT", [C, C], BF16, kind="ExternalInput")
    pwT_d = nc.dram_tensor("pwT", [C, C], BF16, kind="ExternalInput")
    bwT_d = nc.dram_tensor("bwT", [C, H], BF16, kind="ExternalInput")
    projb_d = nc.dram_tensor("projb", [1, C], BF16, kind="ExternalInput")
    boxb_d = nc.dram_tensor("boxb", [1, H], BF16, kind="ExternalInput")
    biasT_d = nc.dram_tensor("biasT", [N, H, N], BF16, kind="ExternalInput")
    out_d = nc.dram_tensor("out", [nb, N, C], F32, kind="ExternalOutput")

    with tile.TileContext(nc) as tc:
        res = tc.alloc_tile_pool(name="res", bufs=1)       # resident singles
        xp = tc.alloc_tile_pool(name="xp", bufs=2)
        qkp = tc.alloc_tile_pool(name="qkp", bufs=2)
        ep = tc.alloc_tile_pool(name="ep", bufs=2)
        ap2 = tc.alloc_tile_pool(name="ap2", bufs=2)
        vp = tc.alloc_tile_pool(name="vp", bufs=2)
        bp = tc.alloc_tile_pool(name="bp", bufs=2)
        sm = tc.alloc_tile_pool(name="sm", bufs=2)         # template smalls
        smr = tc.alloc_tile_pool(name="smr", bufs=4)       # per-head normalize smalls
        ep2 = tc.alloc_tile_pool(name="ep2", bufs=3)       # pre-exp scores staging
        yp = tc.alloc_tile_pool(name="yp", bufs=2)         # output staging
        op = tc.alloc_tile_pool(name="op", bufs=2)
        ps = tc.alloc_tile_pool(name="ps", bufs=3, space="PSUM")
        pt = tc.alloc_tile_pool(name="pt", bufs=2, space="PSUM")
        po = tc.alloc_tile_pool(name="po", bufs=3, space="PSUM")

        # ---------------- preamble: constants + weights ----------------
        ones = res.tile([1, 294], BF16, tag="ones")
        nc.vector.memset(ones, 1.0)
        # row masks for the template->search rescale over j partitions
        maskts = res.tile([1, 128], BF16, tag="maskts")
        nc.vector.memset(maskts, 0.0)
        nc.vector.memset(maskts[0:1, NT:128], 1.0)
        imaskts = res.tile([1, 128], BF16, tag="imaskts")
        nc.vector.memset(imaskts, 1.0)
        nc.vector.memset(imaskts[0:1, NT:128], 0.0)

        wqkT = res.tile([128, 6, 2 * C], BF16, tag="wqkT")
        nc.sync.dma_start(out=wqkT, in_=wqkT_d.rearrange("(cc p) o -> p cc o", p=128))
        wvT = res.tile([128, 6, 2, 384], BF16, tag="wvT")
        nc.scalar.dma_start(out=wvT, in_=wvT_d.rearrange("(cc p) (hf o) -> p cc hf o", p=128, hf=2))
        pwT = res.tile([128, 6, 2, 384], BF16, tag="pwT")
        nc.scalar.dma_start(out=pwT, in_=pwT_d.rearrange("(cc p) (hf o) -> p cc hf o", p=128, hf=2))
        bwT = res.tile([128, 6, H], BF16, tag="bwT")
        nc.scalar.dma_start(out=bwT, in_=bwT_d.rearrange("(cc p) h -> p cc h", p=128))
        projb = boxb = None
        if not VARIANT["zero_bias"]:
            projb = res.tile([1, C], BF16, tag="projb")
            nc.scalar.dma_start(out=projb, in_=projb_d[:])
            boxb = res.tile([1, H], BF16, tag="boxb")
            nc.scalar.dma_start(out=boxb, in_=boxb_d[:])
        # host-gathered RPE bias, transposed layout biasT[ch][j, h, i]
        biasT = []
        for ch, (j0, jw) in enumerate(JCH):
            bt = res.tile([128, H, N], BF16, tag=f"biasT{ch}")
            biasT.append(bt)
            nc.scalar.dma_start(out=bt[0:jw], in_=biasT_d[j0:j0 + jw])

        # ------------- main per-batch-pair loop (software-pipelined) -------
        # Pair p's attn@v + output projection are emitted during iteration
        # p+1, interleaved with pair p+1's qk-projection matmuls, so the PE
        # queue always holds independent work behind each dependency chain.
        assert nb % 2 == 0
        npairs = (nb // 2) * reps

        def stage_front(pair):
            bpair = [2 * pair, 2 * pair + 1]
            st = {"bpair": bpair}
            xpair = xp.tile([128, 6, 2, N], BF16, tag="xpair")
            for t, b in enumerate(bpair):
                nc.sync.dma_start(out=xpair[:, :, t, :],
                                  in_=xT_d[b].rearrange("(cc p) i -> p cc i", p=128))
            bm_ext = bp.tile([NT, 2, H, HD + 1], BF16, tag="bm")
            nc.vector.memset(bm_ext[:, :, :, HD:HD + 1], 1.0)
            for t, b in enumerate(bpair):
                nc.sync.dma_start(out=bm_ext[0:NT, t, :, 0:HD], in_=bm_d[b])
            st["xpair"], st["bm_ext"] = xpair, bm_ext
            st["qkT"] = qkp.tile([128, 12, 2, N], BF16, tag="qkT", name="qkT")
            st["E"] = [ep.tile([128, H, 2, N], BF16, tag=f"E{ch}", name=f"E{ch}")
                       for ch in range(2)]
            st["v_sb"] = vp.tile([128, 2, 2, H, HD + 1], BF16, tag="v", name="v_sb")
            nc.vector.memset(st["v_sb"][:, :, :, :, HD:HD + 1], 1.0)
            st["v_pending"] = True
            st["A"] = [ap2.tile([128, H, 2, NT], BF16, tag=f"A{ch}", name=f"A{ch}")
                       for ch in range(2)]
            return st

        # qk projection (transposed): qkT[p, m, t, i]; m 0..5 = q heads
        # (pre-scaled on host), 6..11 = k heads; head h lives at
        # (m = h//2 (+6 for k), partition half s = h%2).
        def qk_chunk(st, m):
            p_qk = ps.tile([128, 2, N], F32, tag="mm")
            for cc in range(6):
                nc.tensor.matmul(p_qk[:], wqkT[:, cc, 128 * m:128 * m + 128],
                                 st["xpair"][:, cc, :, :],
                                 start=(cc == 0), stop=(cc == 5))
            nc.scalar.copy(st["qkT"][:, m, :, :], p_qk[:])

        def scores_group(st, ch, g):
            qkT, E, A = st["qkT"], st["E"], st["A"]
            j0, jw = JCH[ch]
            tq0, heads = HGROUPS[g]
            hsl = slice(heads[0], min(heads[0] + 6, H), 2)
            epre = ep2.tile([128, 3, 2, N], BF16, tag="epre")
            for u, h in enumerate(heads):
                s, mq, mk = h % 2, h // 2, 6 + h // 2
                p_st = ps.tile([128, 2, N], F32, tag="mm")
                for t in range(2):
                    nc.tensor.matmul(p_st[0:jw, t, :],
                                     qkT[64 * s:64 * s + 64, mk, t, j0:j0 + jw],
                                     qkT[64 * s:64 * s + 64, mq, t, :],
                                     start=True, stop=True)
                bb = biasT[ch][0:jw, h, :].rearrange("p (u i) -> p u i", u=1) \
                    .to_broadcast((jw, 2, N))
                nc.vector.tensor_add(epre[0:jw, u, :, :], p_st[0:jw], bb)
            nc.gpsimd.tensor_copy(A[ch][0:jw, hsl, :, :], epre[0:jw, :, :, 0:NT])
            nc.scalar.activation(E[ch][0:jw, hsl, :, :], epre[0:jw], AF.Exp)

        # template path: out_t^T with denominators from the bm ones column
        def template_group_a(st, g):
            tq0, heads = HGROUPS[g]
            p_ot = pt.tile([65, 3, 2, NT], F32, tag="tiny")
            for u, h in enumerate(heads):
                for t in range(2):
                    nc.tensor.matmul(p_ot[:, u, t, :],
                                     st["bm_ext"][0:NT, t, h, :],
                                     st["E"][0][0:NT, h, t, 0:NT],
                                     start=True, stop=True)
            hsl = slice(heads[0], min(heads[0] + 6, H), 2)
            dtmp = sm.tile([1, 3, 2, NT], F32, tag="dtmp")
            nc.vector.tensor_scalar_add(dtmp, p_ot[64:65, :, :, :], float(N - NT))
            with nc.allow_low_precision("bf16 reciprocal feeds bf16 matmul"):
                nc.vector.reciprocal(st["rTt"][0:1, hsl, :, :], dtmp)
            return p_ot

        def template_group_b(st, g, p_ot):
            tq0, heads = HGROUPS[g]
            sgrp = heads[0] % 2
            hsl = slice(heads[0], min(heads[0] + 6, H), 2)
            rb_t = sm.tile([64, 3, 2, NT], BF16, tag="rbt")
            if VARIANT["attnv_pbcast"]:
                nc.gpsimd.partition_broadcast(rb_t, st["rTt"][0:1, hsl, :, :])
            else:
                p_rbt = po.tile([64, 3, 2, NT], F32, tag="ot")
                nc.tensor.matmul(p_rbt[:], ones[0:1, 0:64],
                                 st["rTt"][0:1, hsl, :, :], start=True, stop=True)
                nc.scalar.copy(rb_t, p_rbt)
            nc.vector.tensor_mul(
                st["otn"][64 * sgrp:64 * sgrp + 64, tq0:tq0 + 3, :, :],
                p_ot[0:64, :, :, :], rb_t)

        def factor_chain(st):
            p_f = pt.tile([H, 2, NT], F32, tag="tiny")
            for cc in range(6):
                nc.tensor.matmul(p_f[:], bwT[:, cc, :], st["otn"][:, cc, :, :],
                                 start=(cc == 0),
                                 stop=(cc == 5 and VARIANT["zero_bias"]))
            if not VARIANT["zero_bias"]:
                nc.tensor.matmul(p_f[:], boxb[0:1, :], ones[0:1, 0:2 * NT],
                                 start=False, stop=True)
            facT = sm.tile([H, 2, NT], BF16, tag="facT")
            nc.scalar.copy(facT, p_f)
            st["facflat"] = sm.tile([1, H, 2, NT], BF16, tag="facflat", name="facflat")
            nc.scalar.dma_start(out=st["facflat"][:], in_=facT[:])

        def v_unit(st, t, ic, hf):
            i0, iw = JCH[ic]
            p_v = ps.tile([128, 384], F32, tag="mm")
            for cc in range(6):
                nc.tensor.matmul(p_v[0:iw], st["xpair"][:, cc, t, i0:i0 + iw],
                                 wvT[:, cc, hf, :],
                                 start=(cc == 0), stop=(cc == 5))
            nc.scalar.copy(
                st["v_sb"][0:iw, ic, t, 6 * hf:6 * hf + 6, 0:HD],
                p_v[0:iw].rearrange("p (r d) -> p r d", r=6))

        # rescale template-query cols of E: E[:, h, :, 0:NT] = exp(A * fb)
        # (chunk0 rows j<NT multiply by 1.0 -> recompute of template block)
        def rescale_group(st, g):
            E, A = st["E"], st["A"]
            tq0, heads = HGROUPS[g]
            hsl = slice(heads[0], min(heads[0] + 6, H), 2)
            fsl = st["facflat"][0:1, hsl, :, :]
            fb0 = pt.tile([128, 3, 2, NT], F32, tag="tiny")
            nc.tensor.matmul(fb0[:], maskts[0:1, :], fsl, start=True, stop=False)
            nc.tensor.matmul(fb0[:], imaskts[0:1, :], ones[0:1, 0:6 * NT],
                             start=False, stop=True)
            nc.vector.tensor_mul(A[0][:, hsl, :, :], A[0][:, hsl, :, :], fb0[:])
            nc.scalar.activation(E[0][:, hsl, :, 0:NT], A[0][:, hsl, :, :], AF.Exp)
            fb1 = pt.tile([128, 3, 2, NT], F32, tag="tiny")
            nc.tensor.matmul(fb1[:], ones[0:1, 0:128], fsl, start=True, stop=True)
            nc.vector.tensor_mul(A[1][0:117, hsl, :, :], A[1][0:117, hsl, :, :],
                                 fb1[0:117])
            nc.scalar.activation(E[1][0:117, hsl, :, 0:NT], A[1][0:117, hsl, :, :],
                                 AF.Exp)

        # attn @ v (denominator = ones row 64) + normalize; the normalize
        # chain (DVE recip -> PE broadcast -> Act copy -> DVE mul) for head
        # h is emitted one head late so PE never waits on it.
        def attnv_head(st, h):
            p_o = po.tile([65, 2, N], F32, tag="ot")
            for t in range(2):
                for ch, (j0, jw) in enumerate(JCH):
                    nc.tensor.matmul(p_o[:, t, :],
                                     st["v_sb"][0:jw, ch, t, h, :],
                                     st["E"][ch][0:jw, h, t, :],
                                     start=(ch == 0), stop=(ch == 1))
            rec = smr.tile([1, 2, N], BF16, tag="rec")
            with nc.allow_low_precision("bf16 reciprocal feeds matmul"):
                nc.vector.reciprocal(rec, p_o[64:65, :, :])
            return (h, p_o, rec)

        def attn_norm(st, h, p_o, rec):
            s, g = h % 2, h // 2
            rb2 = smr.tile([64, 2, N], BF16, tag="rb2")
            if VARIANT["attnv_pbcast"]:
                nc.gpsimd.partition_broadcast(rb2, rec[:])
            else:
                p_rb2 = ps.tile([64, 2, N], F32, tag="mm")
                nc.tensor.matmul(p_rb2[:], ones[0:1, 0:64], rec[0:1, :, :],
                                 start=True, stop=True)
                nc.scalar.copy(rb2, p_rb2)
            nc.vector.tensor_mul(st["OTn"][64 * s:64 * s + 64, g, :, :],
                                 p_o[0:64, :, :], rb2)

        def proj_unit(st, t, ic):
            b = st["bpair"][t]
            i0, iw = JCH[ic]
            y_sb = yp.tile([128, 2, 384], F32, tag="y")
            for hf in range(2):
                p_y = ps.tile([128, 384], F32, tag="mm")
                for cc in range(6):
                    nc.tensor.matmul(p_y[0:iw], st["OTn"][:, cc, t, i0:i0 + iw],
                                     pwT[:, cc, hf, :],
                                     start=(cc == 0),
                                     stop=(cc == 5 and VARIANT["zero_bias"]))
                if not VARIANT["zero_bias"]:
                    nc.tensor.matmul(p_y[0:iw], ones[0:1, 0:iw],
                                     projb[0:1, 384 * hf:384 * hf + 384],
                                     start=False, stop=True)
                nc.vector.tensor_copy(y_sb[0:iw, hf, :], p_y[0:iw])
            nc.scalar.dma_start(out=out_d[b, i0:i0 + iw, :],
                                in_=y_sb[0:iw].rearrange("p hf o -> p (hf o)"))

        def attnv_and_proj(st, interleave=None):
            st["OTn"] = op.tile([128, 6, 2, N], BF16, tag="OTn", name="OTn")
            pending = None
            for g, (_, heads) in enumerate(HGROUPS):
                if interleave is not None:
                    interleave(g)
                for h in heads:
                    nxt = attnv_head(st, h)
                    if pending is not None:
                        attn_norm(st, *pending)
                    pending = nxt
            attn_norm(st, *pending)
            for t in range(2):
                for ic in range(2):
                    proj_unit(st, t, ic)

        def pair_body(st):
            """scores / template / factor / v / rescale for pair st."""
            st["rTt"] = sm.tile([1, H, 2, NT], BF16, tag="rTt", name="rTt")
            st["otn"] = sm.tile([128, 6, 2, NT], BF16, tag="otn", name="otn")
            scores_group(st, 0, 0)
            scores_group(st, 0, 1)
            scores_group(st, 0, 2)
            pot0 = template_group_a(st, 0)
            scores_group(st, 0, 3)
            pot1 = template_group_a(st, 1)
            template_group_b(st, 0, pot0)
            scores_group(st, 1, 0)
            pot2 = template_group_a(st, 2)
            template_group_b(st, 1, pot1)
            scores_group(st, 1, 1)
            pot3 = template_group_a(st, 3)
            template_group_b(st, 2, pot2)
            scores_group(st, 1, 2)
            template_group_b(st, 3, pot3)
            factor_chain(st)
            scores_group(st, 1, 3)
            if st["v_pending"]:
                for t in range(2):
                    for ic in range(2):
                        for hf in range(2):
                            v_unit(st, t, ic, hf)
                st["v_pending"] = False
            for g in range(4):
                rescale_group(st, g)

        prev = None
        for pair0 in range(npairs):
            st = stage_front(pair0 % (nb // 2))
            if prev is None:
                for m in range(12):
                    qk_chunk(st, m)
            else:
                qk_iter = iter(range(12))
                def interleave(g, _st=st, _it=qk_iter):
                    for _ in range(3):
                        m = next(_it, None)
                        if m is not None:
                            qk_chunk(_st, m)
                attnv_and_proj(prev, interleave)
            pair_body(st)
            prev = st
        attnv_and_proj(prev)

        for p in (po, pt, ps, op, yp, ep2, smr, sm, bp, vp, ap2, ep, qkp, xp, res):
            p.release()

    nc.finalize()
    return nc


_CACHE = {}


def _get_nc(nb, reps=1):
    key = (nb, reps, tuple(sorted(VARIANT.items())))
    if key not in _CACHE:
        _CACHE[key] = _build(nb, reps)
    return _CACHE[key]


def _prep_core_inputs(x, boxmask_vec, qkv_w, qkv_b, proj_w, proj_b, box_w, box_b,
                      rpe_table, rpe_index):
    """Host-side prep shared across cores (weights + host RPE gather)."""
    assert np.allclose(qkv_b, 0.0), "kernel assumes qkv_b == 0 (spec fill: zeros)"
    Wq = qkv_w[:C] * np.float32(SCALE)
    Wk = qkv_w[C:2 * C]
    Wv = qkv_w[2 * C:]
    wqkT = np.ascontiguousarray(np.concatenate([Wq, Wk], 0).T).astype(BF)
    wvT = np.ascontiguousarray(Wv.T).astype(BF)
    pwT = np.ascontiguousarray(proj_w.T).astype(BF)
    bwT = np.ascontiguousarray(box_w.T).astype(BF)
    # biasT[j, h, i] = rpe_table[h, rpe_index[i, j]]
    biasT = np.ascontiguousarray(
        np.transpose(rpe_table[:, rpe_index.T], (1, 0, 2))).astype(BF)
    return {
        "wqkT": wqkT, "wvT": wvT, "pwT": pwT, "bwT": bwT,
        "projb": np.ascontiguousarray(proj_b[None, :]).astype(BF),
        "boxb": np.ascontiguousarray(box_b[None, :]).astype(BF),
        "biasT": biasT,
    }


def kernel(x, boxmask_vec, qkv_w, qkv_b, proj_w, proj_b, box_w, box_b,
           rpe_table, rpe_index, lens_t, _nb=NB, _trace=False, _reps=1):
    x = np.asarray(x, np.float32)
    boxmask_vec = np.asarray(boxmask_vec, np.float32)
    qkv_w = np.asarray(qkv_w, np.float32)
    qkv_b = np.asarray(qkv_b, np.float32)
    proj_w = np.asarray(proj_w, np.float32)
    proj_b = np.asarray(proj_b, np.float32)
    box_w = np.asarray(box_w, np.float32)
    box_b = np.asarray(box_b, np.float32)
    rpe_table = np.asarray(rpe_table, np.float32)
    rpe_index = np.asarray(rpe_index, np.int32)
    assert int(lens_t) == NT and x.shape == (B, N, C)

    shared = _prep_core_inputs(x, boxmask_vec, qkv_w, qkv_b, proj_w, proj_b,
                               box_w, box_b, rpe_table, rpe_index)
    VARIANT["zero_bias"] = bool(np.allclose(proj_b, 0.0) and
                                np.allclose(box_b, 0.0))
    nb = _nb
    nc = _get_nc(nb, _reps)
    in_maps = []
    for c in range(NCORES):
        bs = [min(c * nb + i, B - 1) for i in range(nb)]
        m = dict(shared)
        m["xT"] = np.ascontiguousarray(x[bs].transpose(0, 2, 1)).astype(BF)
        m["bm"] = np.ascontiguousarray(
            boxmask_vec[bs].reshape(len(bs), NT, H, HD)).astype(BF)
        in_maps.append(m)
    res = run_bass_kernel_spmd(nc, in_maps, list(range(NCORES)),
                               trace=_trace or bool(os.environ.get("BASS_TRACE")))
    out = np.empty((NCORES * nb, N, C), np.float32)
    for c in range(NCORES):
        out[c * nb:(c + 1) * nb] = res.results[c]["out"]
    if _trace:
        kernel._last = res
    return out[:B] if nb == NB else out



# revision 3
# speedup vs baseline: 1.5299x; 1.5299x over previous
"""Trainium2 Bass kernel for nn_Attention_18631568130798 — v3.

Mixed template/search attention (Stark-style tracker attention).
Sharding: data-parallel over batch B=64 across 8 NeuronCores (8/core).

v3 changes over v2 (HW evidence: PE runs at ~1.2GHz mid p-state regardless
of occupancy; whole-kernel time == sim@1.2GHz to 0.1%, so PE cycles are
the wall and every PE stall is 0.833ns/cycle lost):
  * fb rescale-broadcast matmuls -> Pool partition_broadcast (-3.5k cy/pair)
    with rows<NT of chunk0 left untouched (their E is already correct).
  * proj bias matmuls -> fused into the y_sb eviction as a DVE tensor_add
    against a resident partition-broadcast bias tile (-3.1k cy/pair).
  * factor-chain bias matmul -> DVE tensor_scalar add with per-partition
    boxbT column (-0.1k cy/pair).
  * deeper software pipeline: stage_front(i+1) DMAs issue at the top of
    iteration i; qk and v matmuls of pair i+1 are interleaved as PE filler
    through pair_body(i) (whose scores chain is DVE-paced and used to stall
    the PE ~50%) and attnv+proj(i).
  * out/facflat DMA issuance moved off the Act queue (gpsimd).
"""

import os
import numpy as np
import ml_dtypes

import concourse.bass as bass
import concourse.mybir as mybir
import concourse.tile as tile
from concourse import bacc
from concourse.bass_utils import run_bass_kernel_spmd

AF = mybir.ActivationFunctionType
F32 = mybir.dt.float32
BF16 = mybir.dt.bfloat16
BF = ml_dtypes.bfloat16

B, N, C = 64, 245, 768
H, HD = 12, 64
NT, NS = 49, 196
SCALE = HD ** -0.5
NCORES = 8
NB = B // NCORES
JCH = [(0, 128), (128, 117)]
HGROUPS = [(0, [0, 2, 4]), (0, [1, 3, 5]), (3, [6, 8, 10]), (3, [7, 9, 11])]


def _build(nb: int, reps: int = 1):
    nc = bacc.Bacc(None, target_bir_lowering=False, name="attn18631v6")

    xT_d = nc.dram_tensor("xT", [nb, C, N], BF16, kind="ExternalInput")
    bm_d = nc.dram_tensor("bm", [nb, NT, H, HD], BF16, kind="ExternalInput")
    wqkT_d = nc.dram_tensor("wqkT", [C, 2 * C], BF16, kind="ExternalInput")
    wvT_d = nc.dram_tensor("wvT", [C, C], BF16, kind="ExternalInput")
    pwT_d = nc.dram_tensor("pwT", [C, C], BF16, kind="ExternalInput")
    bwT_d = nc.dram_tensor("bwT", [C, H], BF16, kind="ExternalInput")
    projb_d = nc.dram_tensor("projb", [1, C], BF16, kind="ExternalInput")
    boxbT_d = nc.dram_tensor("boxbT", [H, 1], F32, kind="ExternalInput")
    biasT_d = nc.dram_tensor("biasT", [N, H, N], BF16, kind="ExternalInput")
    out_d = nc.dram_tensor("out", [nb, N, C], F32, kind="ExternalOutput")

    with tile.TileContext(nc) as tc:
        res = tc.alloc_tile_pool(name="res", bufs=1)
        xp = tc.alloc_tile_pool(name="xp", bufs=2)
        qkp = tc.alloc_tile_pool(name="qkp", bufs=2)
        ep = tc.alloc_tile_pool(name="ep", bufs=2)
        ap2 = tc.alloc_tile_pool(name="ap2", bufs=2)
        vp = tc.alloc_tile_pool(name="vp", bufs=2)
        bp = tc.alloc_tile_pool(name="bp", bufs=2)
        sm = tc.alloc_tile_pool(name="sm", bufs=2)
        smr = tc.alloc_tile_pool(name="smr", bufs=4)
        ep2 = tc.alloc_tile_pool(name="ep2", bufs=3)
        yp = tc.alloc_tile_pool(name="yp", bufs=2)
        op = tc.alloc_tile_pool(name="op", bufs=1)
        ps = tc.alloc_tile_pool(name="ps", bufs=3, space="PSUM")
        pt = tc.alloc_tile_pool(name="pt", bufs=2, space="PSUM")
        po = tc.alloc_tile_pool(name="po", bufs=3, space="PSUM")

        # ---------------- preamble: constants + weights ----------------
        ones = res.tile([1, 294], BF16, tag="ones")
        nc.vector.memset(ones, 1.0)

        # wqkT split 3 ways across DMA queues so qk(0) can start ~2.5us in
        wqkT3 = []
        for k, eng in enumerate((nc.sync, nc.scalar, nc.gpsimd)):
            wt = res.tile([128, 2, 2 * C], BF16, tag=f"wqkT{k}", name=f"wqkT{k}")
            wqkT3.append(wt)
            eng.dma_start(out=wt, in_=wqkT_d.rearrange(
                "(cc p) o -> p cc o", p=128)[:, 2 * k:2 * k + 2, :])
        wvT = res.tile([128, 6, 2, 384], BF16, tag="wvT")
        nc.scalar.dma_start(out=wvT, in_=wvT_d.rearrange("(cc p) (hf o) -> p cc hf o", p=128, hf=2))
        pwT = res.tile([128, 6, 2, 384], BF16, tag="pwT")
        nc.sync.dma_start(out=pwT, in_=pwT_d.rearrange("(cc p) (hf o) -> p cc hf o", p=128, hf=2))
        bwT = res.tile([128, 6, H], BF16, tag="bwT")
        projb = res.tile([1, C], BF16, tag="projb")
        projb_bc = res.tile([128, C], BF16, tag="projb_bc")
        boxbT = res.tile([H, 1], F32, tag="boxbT")

        def preamble_tail():
            nc.gpsimd.dma_start(out=bwT, in_=bwT_d.rearrange("(cc p) h -> p cc h", p=128))
            nc.gpsimd.dma_start(out=projb, in_=projb_d[:])
            nc.gpsimd.partition_broadcast(projb_bc, projb)
            nc.gpsimd.dma_start(out=boxbT, in_=boxbT_d[:])
        biasT = []
        for ch, (j0, jw) in enumerate(JCH):
            bt = res.tile([128, H, N], BF16, tag=f"biasT{ch}")
            biasT.append(bt)
            (nc.scalar if ch == 0 else nc.sync).dma_start(
                out=bt[0:jw], in_=biasT_d[j0:j0 + jw])

        assert nb % 2 == 0
        npairs = (nb // 2) * reps

        def stage_front(pair):
            bpair = [2 * pair, 2 * pair + 1]
            st = {"bpair": bpair}
            xpair = xp.tile([128, 6, 2, N], BF16, tag="xpair")
            for t, b in enumerate(bpair):
                nc.gpsimd.dma_start(out=xpair[:, :, t, :],
                                    in_=xT_d[b].rearrange("(cc p) i -> p cc i", p=128))
            bm_ext = bp.tile([NT, 2, H, HD + 1], BF16, tag="bm")
            nc.vector.memset(bm_ext[:, :, :, HD:HD + 1], 1.0)
            for t, b in enumerate(bpair):
                nc.sync.dma_start(out=bm_ext[0:NT, t, :, 0:HD], in_=bm_d[b])
            st["xpair"], st["bm_ext"] = xpair, bm_ext
            st["qkT"] = qkp.tile([128, 12, 2, N], BF16, tag="qkT", name="qkT")
            st["E"] = [ep.tile([128, H, 2, N], BF16, tag=f"E{ch}", name=f"E{ch}")
                       for ch in range(2)]
            st["v_sb"] = vp.tile([128, 2, 2, H, HD + 1], BF16, tag="v", name="v_sb")
            nc.vector.memset(st["v_sb"][:, :, :, :, HD:HD + 1], 1.0)
            st["A"] = [ap2.tile([128, H, 2, NT], BF16, tag=f"A{ch}", name=f"A{ch}")
                       for ch in range(2)]
            return st

        # qk projection (transposed): qkT[p, m, t, i]; m 0..5 = q heads
        # (pre-scaled on host), 6..11 = k heads; head h lives at
        # (m = h//2 (+6 for k), partition half s = h%2).
        def qk_chunk(st, m):
            p_qk = ps.tile([128, 2, N], F32, tag="mm")
            for cc in range(6):
                nc.tensor.matmul(p_qk[:],
                                 wqkT3[cc // 2][:, cc % 2, 128 * m:128 * m + 128],
                                 st["xpair"][:, cc, :, :],
                                 start=(cc == 0), stop=(cc == 5))
            nc.scalar.copy(st["qkT"][:, m, :, :], p_qk[:])

        def v_unit(st, t, ic, hf):
            i0, iw = JCH[ic]
            p_v = ps.tile([128, 384], F32, tag="mm")
            for cc in range(6):
                nc.tensor.matmul(p_v[0:iw], st["xpair"][:, cc, t, i0:i0 + iw],
                                 wvT[:, cc, hf, :],
                                 start=(cc == 0), stop=(cc == 5))
            nc.scalar.copy(
                st["v_sb"][0:iw, ic, t, 6 * hf:6 * hf + 6, 0:HD],
                p_v[0:iw].rearrange("p (r d) -> p r d", r=6))

        def scores_group(st, ch, g):
            qkT, E, A = st["qkT"], st["E"], st["A"]
            j0, jw = JCH[ch]
            tq0, heads = HGROUPS[g]
            hsl = slice(heads[0], min(heads[0] + 6, H), 2)
            epre = ep2.tile([128, 3, 2, N], BF16, tag="epre")
            for u, h in enumerate(heads):
                s, mq, mk = h % 2, h // 2, 6 + h // 2
                p_st = ps.tile([128, 2, N], F32, tag="mm")
                for t in range(2):
                    nc.tensor.matmul(p_st[0:jw, t, :],
                                     qkT[64 * s:64 * s + 64, mk, t, j0:j0 + jw],
                                     qkT[64 * s:64 * s + 64, mq, t, :],
                                     start=True, stop=True)
                bb = biasT[ch][0:jw, h, :].rearrange("p (u i) -> p u i", u=1) \
                    .to_broadcast((jw, 2, N))
                nc.vector.tensor_add(epre[0:jw, u, :, :], p_st[0:jw], bb)
            nc.gpsimd.tensor_copy(A[ch][0:jw, hsl, :, :], epre[0:jw, :, :, 0:NT])
            nc.scalar.activation(E[ch][0:jw, hsl, :, :], epre[0:jw], AF.Exp)

        # template path: out_t^T with denominators from the bm ones column
        def template_group_a(st, g):
            tq0, heads = HGROUPS[g]
            p_ot = pt.tile([65, 3, 2, NT], F32, tag="tiny")
            for u, h in enumerate(heads):
                for t in range(2):
                    nc.tensor.matmul(p_ot[:, u, t, :],
                                     st["bm_ext"][0:NT, t, h, :],
                                     st["E"][0][0:NT, h, t, 0:NT],
                                     start=True, stop=True)
            dtmp = sm.tile([1, 3, 2, NT], F32, tag="dtmp")
            nc.vector.tensor_scalar_add(dtmp, p_ot[64:65, :, :, :], float(N - NT))
            with nc.allow_low_precision("bf16 reciprocal feeds bf16 matmul"):
                nc.vector.reciprocal(st["rTt"][0:1, g], dtmp)
            return p_ot

        def template_group_b(st, g, p_ot):
            tq0, heads = HGROUPS[g]
            sgrp = heads[0] % 2
            rb_t = sm.tile([64, 3, 2, NT], BF16, tag="rbt")
            nc.gpsimd.partition_broadcast(rb_t, st["rTt"][0:1, g])
            nc.vector.tensor_mul(
                st["otn"][64 * sgrp:64 * sgrp + 64, tq0:tq0 + 3, :, :],
                p_ot[0:64, :, :, :], rb_t)

        def factor_chain(st):
            p_f = pt.tile([H, 2, NT], F32, tag="tiny")
            for cc in range(6):
                nc.tensor.matmul(p_f[:], bwT[:, cc, :], st["otn"][:, cc, :, :],
                                 start=(cc == 0), stop=(cc == 5))
            facT = sm.tile([H, 2, NT], BF16, tag="facT")
            nc.vector.tensor_scalar_add(facT, p_f, boxbT[:, 0:1])
            st["facflat"] = sm.tile([1, H, 2, NT], BF16, tag="facflat", name="facflat")
            nc.gpsimd.dma_start(out=st["facflat"][:], in_=facT[:])

        # rescale template-query cols of E for search rows:
        # E[j, h, t, i<NT] = exp(A * fac) for j >= NT (chunk0 rows < NT keep
        # factor 1.0 and their E entries are already correct from scores).
        def rescale_group(st, g):
            E, A = st["E"], st["A"]
            tq0, heads = HGROUPS[g]
            hsl = slice(heads[0], min(heads[0] + 6, H), 2)
            fsl = st["facflat"][0:1, 3 * g:3 * g + 3, :, :]
            rb = smr.tile([128, 3, 2, NT], BF16, tag="rbf")
            nc.gpsimd.partition_broadcast(rb, fsl)
            nc.vector.tensor_mul(A[1][0:117, hsl, :, :], A[1][0:117, hsl, :, :],
                                 rb[0:117])
            nc.scalar.activation(E[1][0:117, hsl, :, 0:NT], A[1][0:117, hsl, :, :],
                                 AF.Exp)
            nc.vector.memset(rb[0:NT], 1.0)
            nc.vector.tensor_mul(A[0][:, hsl, :, :], A[0][:, hsl, :, :], rb)
            nc.scalar.activation(E[0][:, hsl, :, 0:NT], A[0][:, hsl, :, :],
                                 AF.Exp)

        # attn @ v (denominator = ones row 64) + normalize; normalize chain
        # emitted one head late so PE never waits on it.
        def attnv_head(st, h):
            p_o = po.tile([65, 2, N], F32, tag="ot")
            for t in range(2):
                for ch, (j0, jw) in enumerate(JCH):
                    nc.tensor.matmul(p_o[:, t, :],
                                     st["v_sb"][0:jw, ch, t, h, :],
                                     st["E"][ch][0:jw, h, t, :],
                                     start=(ch == 0), stop=(ch == 1))
            rec = smr.tile([1, 2, N], BF16, tag="rec")
            with nc.allow_low_precision("bf16 reciprocal feeds matmul"):
                nc.vector.reciprocal(rec, p_o[64:65, :, :])
            return (h, p_o, rec)

        def attn_norm(st, h, p_o, rec):
            s, g = h % 2, h // 2
            rb2 = smr.tile([64, 2, N], BF16, tag="rb2")
            nc.gpsimd.partition_broadcast(rb2, rec[:])
            nc.vector.tensor_mul(st["OTn"][64 * s:64 * s + 64, g, :, :],
                                 p_o[0:64, :, :], rb2)

        def proj_unit(st, t, ic):
            b = st["bpair"][t]
            i0, iw = JCH[ic]
            y_sb = yp.tile([128, 2, 384], F32, tag="y")
            for hf in range(2):
                p_y = ps.tile([128, 384], F32, tag="mm")
                for cc in range(6):
                    nc.tensor.matmul(p_y[0:iw], st["OTn"][:, cc, t, i0:i0 + iw],
                                     pwT[:, cc, hf, :],
                                     start=(cc == 0), stop=(cc == 5))
                nc.vector.tensor_add(y_sb[0:iw, hf, :], p_y[0:iw],
                                     projb_bc[0:iw, 384 * hf:384 * hf + 384])
            nc.gpsimd.dma_start(out=out_d[b, i0:i0 + iw, :],
                                in_=y_sb[0:iw].rearrange("p hf o -> p (hf o)"))

        # ---- software pipeline ----
        # Iteration i: body(i) + attnv/proj(i), with qk(i+1) and v(i+1)
        # matmuls woven in as PE filler (they only need xpair(i+1), whose
        # DMA is issued at the top of iteration i).
        def make_filler(st_next):
            work = []
            if st_next is not None:
                work += [lambda m=m: qk_chunk(st_next, m) for m in range(12)]
                work += [lambda t=t, ic=ic, hf=hf: v_unit(st_next, t, ic, hf)
                         for t in range(2) for ic in range(2) for hf in range(2)]
            it = iter(work)

            def fill(k):
                for _ in range(k):
                    w = next(it, None)
                    if w is None:
                        return
                    w()
            return fill

        def pair_body(st, fill):
            st["rTt"] = sm.tile([1, 4, 3, 2, NT], BF16, tag="rTt", name="rTt")
            st["otn"] = sm.tile([128, 6, 2, NT], BF16, tag="otn", name="otn")
            scores_group(st, 0, 0)
            scores_group(st, 0, 1)
            fill(1)
            scores_group(st, 0, 2)
            pot0 = template_group_a(st, 0)
            fill(1)
            scores_group(st, 0, 3)
            pot1 = template_group_a(st, 1)
            template_group_b(st, 0, pot0)
            fill(1)
            scores_group(st, 1, 0)
            pot2 = template_group_a(st, 2)
            template_group_b(st, 1, pot1)
            fill(1)
            scores_group(st, 1, 1)
            pot3 = template_group_a(st, 3)
            template_group_b(st, 2, pot2)
            fill(1)
            scores_group(st, 1, 2)
            template_group_b(st, 3, pot3)
            factor_chain(st)
            fill(1)
            scores_group(st, 1, 3)
            fill(2)
            rescale_group(st, 0)
            fill(1)
            rescale_group(st, 1)
            fill(1)
            rescale_group(st, 2)
            fill(1)
            rescale_group(st, 3)

        def attnv_and_proj(st, fill):
            st["OTn"] = op.tile([128, 6, 2, N], BF16, tag="OTn", name="OTn")
            pending = None
            for g, (_, heads) in enumerate(HGROUPS):
                fill(2)
                for h in heads:
                    nxt = attnv_head(st, h)
                    if pending is not None:
                        attn_norm(st, *pending)
                    pending = nxt
            attn_norm(st, *pending)
            for t in range(2):
                for ic in range(2):
                    fill(1)
                    proj_unit(st, t, ic)
            fill(100)

        st = stage_front(0)
        preamble_tail()
        for m in range(12):
            qk_chunk(st, m)
        for t in range(2):
            for ic in range(2):
                for hf in range(2):
                    v_unit(st, t, ic, hf)
        for i in range(npairs):
            st_next = stage_front((i + 1) % (nb // 2)) if i + 1 < npairs else None
            fill = make_filler(st_next)
            pair_body(st, fill)
            attnv_and_proj(st, fill)
            st = st_next

        for p in (po, pt, ps, op, yp, ep2, smr, sm, bp, vp, ap2, ep, qkp, xp, res):
            p.release()

    nc.finalize()
    return nc


_CACHE = {}


def _get_nc(nb, reps=1):
    key = (nb, reps)
    if key not in _CACHE:
        _CACHE[key] = _build(nb, reps)
    return _CACHE[key]


def _prep_core_inputs(x, boxmask_vec, qkv_w, qkv_b, proj_w, proj_b, box_w, box_b,
                      rpe_table, rpe_index):
    """Host-side prep shared across cores (weights + host RPE gather)."""
    assert np.allclose(qkv_b, 0.0), "kernel assumes qkv_b == 0 (spec fill: zeros)"
    Wq = qkv_w[:C] * np.float32(SCALE)
    Wk = qkv_w[C:2 * C]
    Wv = qkv_w[2 * C:]
    wqkT = np.ascontiguousarray(np.concatenate([Wq, Wk], 0).T).astype(BF)
    wvT = np.ascontiguousarray(Wv.T).astype(BF)
    pwT = np.ascontiguousarray(proj_w.T).astype(BF)
    perm = [0, 2, 4, 1, 3, 5, 6, 8, 10, 7, 9, 11]
    bwT = np.ascontiguousarray(box_w.T[:, perm]).astype(BF)
    biasT = np.ascontiguousarray(
        np.transpose(rpe_table[:, rpe_index.T], (1, 0, 2))).astype(BF)
    return {
        "wqkT": wqkT, "wvT": wvT, "pwT": pwT, "bwT": bwT,
        "projb": np.ascontiguousarray(proj_b[None, :]).astype(BF),
        "boxbT": np.ascontiguousarray(box_b[perm, None]).astype(np.float32),
        "biasT": biasT,
    }


def kernel(x, boxmask_vec, qkv_w, qkv_b, proj_w, proj_b, box_w, box_b,
           rpe_table, rpe_index, lens_t, _nb=NB, _trace=False, _reps=1):
    x = np.asarray(x, np.float32)
    boxmask_vec = np.asarray(boxmask_vec, np.float32)
    qkv_w = np.asarray(qkv_w, np.float32)
    qkv_b = np.asarray(qkv_b, np.float32)
    proj_w = np.asarray(proj_w, np.float32)
    proj_b = np.asarray(proj_b, np.float32)
    box_w = np.asarray(box_w, np.float32)
    box_b = np.asarray(box_b, np.float32)
    rpe_table = np.asarray(rpe_table, np.float32)
    rpe_index = np.asarray(rpe_index, np.int32)
    assert int(lens_t) == NT and x.shape == (B, N, C)

    shared = _prep_core_inputs(x, boxmask_vec, qkv_w, qkv_b, proj_w, proj_b,
                               box_w, box_b, rpe_table, rpe_index)
    nb = _nb
    nc = _get_nc(nb, _reps)
    in_maps = []
    for c in range(NCORES):
        bs = [min(c * nb + i, B - 1) for i in range(nb)]
        m = dict(shared)
        m["xT"] = np.ascontiguousarray(x[bs].transpose(0, 2, 1)).astype(BF)
        m["bm"] = np.ascontiguousarray(
            boxmask_vec[bs].reshape(len(bs), NT, H, HD)).astype(BF)
        in_maps.append(m)
    res = run_bass_kernel_spmd(nc, in_maps, list(range(NCORES)),
                               trace=_trace or bool(os.environ.get("BASS_TRACE")))
    out = np.empty((NCORES * nb, N, C), np.float32)
    for c in range(NCORES):
        out[c * nb:(c + 1) * nb] = res.results[c]["out"]
    if _trace:
        kernel._last = res
    return out[:B] if nb == NB else out
